# revision 9
# baseline (speedup 1.0000x reference)
"""2-layer GCN (PyG GCNConv x2 + ReLU) on 8 Trainium2 NeuronCores.

Math (per layer, A = adjacency from edge_index, D = deg(dst)+1 with self loops):
    h   = x @ W
    out = relu(dis * (A @ (dis*h) + dis*h) + b),   dis = D^{-1/2}

Sharding: nodes are packed into 128-node "blocks" balanced by in-degree
(snake assignment over degree-sorted nodes).  Each of the 8 cores owns
NB blocks.  Per layer: each core computes h' = (x@W)*dis for its nodes,
the h' tables are AllGathered (bf16), and each core aggregates messages
for its own dst blocks by indirect-DMA gathering h'[src] rows and
summing them with one-hot matmuls accumulated in PSUM.
"""

import math

import ml_dtypes
import numpy as np

import concourse.bass as bass
import concourse.mybir as mybir
import concourse.tile as tile
from concourse.bass_utils import run_bass_kernel_spmd
from concourse.masks import make_identity
from concourse.vector_clock import ScopedClock

P = 128
NCORES = 8
F32 = mybir.dt.float32
BF16 = mybir.dt.bfloat16
I32 = mybir.dt.int32
PAD_LANE = 1000.0  # dst-lane sentinel for padding edge slots (one-hot = 0)


def _patched_drain_and_barrier(self, tick_clock, wait_clock):
    # This walrus build rejects >1 sem wait on TPB_CTRL (Drain) instructions.
    # Spill the tile-epilogue drain waits onto extra single-wait drains.
    drain_inst = self.nc.sync.drain()
    wait_clock.add_sem_waits(
        drain_inst.ins, ScopedClock({None: tick_clock.global_clock})
    )
    si = drain_inst.ins.sync_info
    waits = list(si.on_wait)
    if len(waits) > 1:
        while len(si.on_wait):
            si.on_wait.pop()
        si.on_wait.append(waits[0])
        for w in waits[1:]:
            d2 = self.nc.sync.drain(fusable=False)
            si2 = d2.ins.sync_info
            if si2 is None:
                d2.ins.sync_info = mybir.SyncInfo(on_wait=[w], on_update=[])
            else:
                si2.on_wait.append(w)
    self.nc.all_engine_barrier()
    popped = self.nc._tile_sem_poison_stack.pop()
    assert popped is self._sem_poison
    self.nc.clear_and_free_semaphores(list(self.sems.allocated().values()))
    self.nc.all_engine_barrier()


tile.TileContext._drain_and_barrier = _patched_drain_and_barrier


def _spill_waits(nc, max_waits=1):
    """This walrus build accepts at most one sync wait per instruction.
    Move extra waits onto dedicated single-wait NoOps ahead of the
    instruction on the same engine (engines execute in program order)."""
    n = 0
    for f in nc.m.functions:
        for blk in f.blocks:
            il = blk.instructions
            out = []
            for inst in il:
                si = inst.sync_info
                if si is not None and len(si.on_wait) > max_waits:
                    waits = list(si.on_wait)
                    while len(si.on_wait):
                        si.on_wait.pop()
                    for w in waits[:max_waits]:
                        si.on_wait.append(w)
                    for w in waits[max_waits:]:
                        nop = mybir.InstNoOp(
                            name=f"waitspill-{n}",
                            sync_info=mybir.SyncInfo(on_wait=[w], on_update=[]),
                            bass_nofuse=True,
                            engine=inst.engine,
                        )
                        n += 1
                        out.append(nop)
                out.append(inst)
            blk.instructions = out
    return n


def _build_program(NB, CPB, IN_CH, HID, OUT_CH, has_b1, has_b2):
    """One SPMD program; per-core data comes via input tensors."""
    NPC = NB * P  # nodes per core
    NTOT = NPC * NCORES  # rows in the allgathered tables
    KT = IN_CH // P  # k-tiles for the layer-1 dense matmul
    assert IN_CH % P == 0 and HID <= 512 and OUT_CH <= 512

    nc = bass.Bass()
    xT = nc.dram_tensor("xT", [IN_CH, NPC], F32, kind="ExternalInput")
    W1 = nc.dram_tensor("W1", [IN_CH, HID], F32, kind="ExternalInput")
    W2 = nc.dram_tensor("W2", [HID, OUT_CH], F32, kind="ExternalInput")
    b1bc = nc.dram_tensor("b1bc", [P, HID], F32, kind="ExternalInput")
    b2bc = nc.dram_tensor("b2bc", [P, OUT_CH], F32, kind="ExternalInput")
    disT = nc.dram_tensor("disT", [P, NB], F32, kind="ExternalInput")
    esrc = nc.dram_tensor("esrc", [P, NB * CPB], I32, kind="ExternalInput")
    dstl = nc.dram_tensor("dstl", [P, NB * CPB], F32, kind="ExternalInput")
    outY = nc.dram_tensor("outY", [NPC, OUT_CH], F32, kind="ExternalOutput")

    h1s = nc.dram_tensor("h1s", [NPC, HID], BF16)
    h1f = nc.dram_tensor("h1f", [NTOT, HID], BF16)
    h2s = nc.dram_tensor("h2s", [NPC, OUT_CH], BF16)
    h2f = nc.dram_tensor("h2f", [NTOT, OUT_CH], BF16)

    rg = [list(range(NCORES))]
    RELU = mybir.ActivationFunctionType.Relu
    ADD = mybir.AluOpType.add
    ISEQ = mybir.AluOpType.is_equal

    with tile.TileContext(nc) as tc:
        with tc.tile_pool(name="const", bufs=1) as cst:
            w1sb = cst.tile([P, KT * HID], F32)
            for k in range(KT):
                nc.sync.dma_start(
                    out=w1sb[:, k * HID : (k + 1) * HID], in_=W1[k * P : (k + 1) * P, :]
                )
            w2sb = cst.tile([P, OUT_CH], F32)
            nc.sync.dma_start(out=w2sb[:], in_=W2[:, :])
            b1sb = cst.tile([P, HID], F32)
            nc.sync.dma_start(out=b1sb[:], in_=b1bc[:, :])
            b2sb = cst.tile([P, OUT_CH], F32)
            nc.sync.dma_start(out=b2sb[:], in_=b2bc[:, :])
            dissb = cst.tile([P, NB], F32)
            nc.sync.dma_start(out=dissb[:], in_=disT[:, :])
            esrcsb = cst.tile([P, NB * CPB], I32)
            nc.sync.dma_start(out=esrcsb[:], in_=esrc[:, :])
            dstlsb = cst.tile([P, NB * CPB], F32)
            nc.sync.dma_start(out=dstlsb[:], in_=dstl[:, :])
            iotasb = cst.tile([P, P], BF16)
            nc.gpsimd.iota(
                iotasb[:],
                pattern=[[1, P]],
                base=0,
                channel_multiplier=0,
                allow_small_or_imprecise_dtypes=True,
            )
            idsb = cst.tile([P, P], F32)
            make_identity(nc, idsb[:])
            h1p = cst.tile([P, NB * HID], F32)  # h' shard, layer 1 (f32)
            h2p = cst.tile([P, NB * OUT_CH], F32)  # h' shard, layer 2 (f32)

            # ---- Phase A: h1' = (x @ W1) * dis  (per block)
            with (
                tc.tile_pool(name="pa", bufs=3) as pa,
                tc.tile_pool(name="pap", bufs=2, space="PSUM") as pap,
            ):
                for b in range(NB):
                    xt = pa.tile([P, KT * P], F32, tag="xt")
                    for k in range(KT):
                        nc.sync.dma_start(
                            out=xt[:, k * P : (k + 1) * P],
                            in_=xT[k * P : (k + 1) * P, b * P : (b + 1) * P],
                        )
                    ps0 = pap.tile([P, HID], F32, tag="ps0")
                    for k in range(KT):
                        nc.tensor.matmul(
                            ps0[:],
                            lhsT=xt[:, k * P : (k + 1) * P],
                            rhs=w1sb[:, k * HID : (k + 1) * HID],
                            start=(k == 0),
                            stop=(k == KT - 1),
                        )
                    nc.vector.tensor_scalar_mul(
                        h1p[:, b * HID : (b + 1) * HID], ps0[:], dissb[:, b : b + 1]
                    )

            # ---- Phase B: shard -> DRAM (cast bf16), AllGather
            nc.gpsimd.dma_start(
                out=h1s[:, :].rearrange("(b p) f -> p b f", p=P),
                in_=h1p[:].rearrange("p (b f) -> p b f", f=HID),
            )
            nc.gpsimd.collective_compute(
                "AllGather",
                mybir.AluOpType.bypass,
                replica_groups=rg,
                ins=[h1s[:, :]],
                outs=[h1f[:, :]],
            )

            # ---- Phase C: aggregate layer 1, layer-1 epilogue, h2' = (out1@W2)*dis
            with (
                tc.tile_pool(name="pc", bufs=3) as pc,
                tc.tile_pool(name="pcm", bufs=4) as pcm,
                tc.tile_pool(name="pcp", bufs=2, space="PSUM") as pcp,
                tc.tile_pool(name="pcq", bufs=2, space="PSUM") as pcq,
            ):
                for b in range(NB):
                    g1 = pc.tile([P, CPB * HID], BF16, tag="g1")
                    for c in range(CPB):
                        col = b * CPB + c
                        nc.gpsimd.indirect_dma_start(
                            out=g1[:, c * HID : (c + 1) * HID],
                            out_offset=None,
                            in_=h1f[:, :],
                            in_offset=bass.IndirectOffsetOnAxis(
                                ap=esrcsb[:, col : col + 1], axis=0
                            ),
                        )
                    ps1 = pcp.tile([P, HID], F32, tag="ps1")
                    for c in range(CPB):
                        m = pcm.tile([P, P], BF16, tag="m")
                        col = b * CPB + c
                        nc.vector.tensor_scalar(
                            m[:], iotasb[:], dstlsb[:, col : col + 1], None, ISEQ
                        )
                        nc.tensor.matmul(
                            ps1[:],
                            lhsT=m[:],
                            rhs=g1[:, c * HID : (c + 1) * HID],
                            start=(c == 0),
                            stop=(c == CPB - 1),
                        )
                    t0 = pc.tile([P, HID], F32, tag="t0")
                    nc.vector.tensor_tensor(
                        t0[:], ps1[:], h1p[:, b * HID : (b + 1) * HID], op=ADD
                    )
                    o1 = pc.tile([P, HID], F32, tag="o1")
                    if has_b1:
                        nc.vector.tensor_scalar_mul(t0[:], t0[:], dissb[:, b : b + 1])
                        nc.vector.tensor_tensor(t0[:], t0[:], b1sb[:], op=ADD)
                        nc.scalar.activation(o1[:], t0[:], RELU)
                    else:
                        nc.scalar.activation(
                            o1[:], t0[:], RELU, scale=dissb[:, b : b + 1]
                        )
                    pst = pcq.tile([P, HID], F32, tag="pst")
                    nc.tensor.transpose(out=pst[:], in_=o1[:], identity=idsb[:])
                    o1t = pc.tile([P, HID], F32, tag="o1t")
                    nc.scalar.copy(out=o1t[:], in_=pst[:])
                    ps2 = pcq.tile([P, OUT_CH], F32, tag="ps2")
                    nc.tensor.matmul(
                        ps2[:], lhsT=o1t[:], rhs=w2sb[:], start=True, stop=True
                    )
                    nc.vector.tensor_scalar_mul(
                        h2p[:, b * OUT_CH : (b + 1) * OUT_CH],
                        ps2[:],
                        dissb[:, b : b + 1],
                    )

            # ---- Phase D: shard -> DRAM (cast bf16), AllGather
            nc.gpsimd.dma_start(
                out=h2s[:, :].rearrange("(b p) f -> p b f", p=P),
                in_=h2p[:].rearrange("p (b f) -> p b f", f=OUT_CH),
            )
            nc.gpsimd.collective_compute(
                "AllGather",
                mybir.AluOpType.bypass,
                replica_groups=rg,
                ins=[h2s[:, :]],
                outs=[h2f[:, :]],
            )

            # ---- Phase E: aggregate layer 2, final epilogue, write output
            with (
                tc.tile_pool(name="pe", bufs=3) as pe,
                tc.tile_pool(name="pem", bufs=4) as pem,
                tc.tile_pool(name="pep", bufs=2, space="PSUM") as pep,
            ):
                for b in range(NB):
                    g2 = pe.tile([P, CPB * OUT_CH], BF16, tag="g2")
                    for c in range(CPB):
                        col = b * CPB + c
                        nc.gpsimd.indirect_dma_start(
                            out=g2[:, c * OUT_CH : (c + 1) * OUT_CH],
                            out_offset=None,
                            in_=h2f[:, :],
                            in_offset=bass.IndirectOffsetOnAxis(
                                ap=esrcsb[:, col : col + 1], axis=0
                            ),
                        )
                    ps3 = pep.tile([P, OUT_CH], F32, tag="ps3")
                    for c in range(CPB):
                        m2 = pem.tile([P, P], BF16, tag="m2")
                        col = b * CPB + c
                        nc.vector.tensor_scalar(
                            m2[:], iotasb[:], dstlsb[:, col : col + 1], None, ISEQ
                        )
                        nc.tensor.matmul(
                            ps3[:],
                            lhsT=m2[:],
                            rhs=g2[:, c * OUT_CH : (c + 1) * OUT_CH],
                            start=(c == 0),
                            stop=(c == CPB - 1),
                        )
                    t2 = pe.tile([P, OUT_CH], F32, tag="t2")
                    nc.vector.tensor_tensor(
                        t2[:], ps3[:], h2p[:, b * OUT_CH : (b + 1) * OUT_CH], op=ADD
                    )
                    o2 = pe.tile([P, OUT_CH], F32, tag="o2")
                    if has_b2:
                        nc.vector.tensor_scalar_mul(t2[:], t2[:], dissb[:, b : b + 1])
                        nc.vector.tensor_tensor(t2[:], t2[:], b2sb[:], op=ADD)
                        nc.scalar.activation(o2[:], t2[:], RELU)
                    else:
                        nc.scalar.activation(
                            o2[:], t2[:], RELU, scale=dissb[:, b : b + 1]
                        )
                    nc.sync.dma_start(out=outY[b * P : (b + 1) * P, :], in_=o2[:])

    _spill_waits(nc)
    return nc


def _prepare(x, src, dst):
    """Host-side sharding: degree-balanced node->block assignment + edge slots."""
    N = x.shape[0]
    E = src.shape[0]
    NB = int(math.ceil(N / (NCORES * P)))  # blocks per core
    TB = NB * NCORES  # total blocks
    NPC = NB * P
    NTOT = NPC * NCORES

    indeg = np.bincount(dst, minlength=N).astype(np.int64)
    dis = (1.0 / np.sqrt(indeg.astype(np.float32) + 1.0)).astype(np.float32)

    # Snake assignment of degree-sorted nodes over TB blocks -> balanced
    # per-block edge counts; round r = lane r (<=128 rounds by construction).
    order = np.argsort(-indeg, kind="stable")
    i = np.arange(N)
    rnd = i // TB
    pos = i % TB
    blk_i = np.where(rnd % 2 == 0, pos, TB - 1 - pos)
    assert rnd.max() < P
    gid_of = np.empty(N, np.int64)
    gid_of[order] = blk_i * P + rnd
    node_of_gid = np.full(NTOT, -1, np.int64)
    node_of_gid[gid_of] = np.arange(N)

    # Edge slots: group edges by dst block; slot (chunk, lane) within block.
    gdst = gid_of[dst]
    eblk = gdst >> 7
    eord = np.argsort(eblk, kind="stable")
    eblk_s = eblk[eord]
    counts = np.bincount(eblk_s, minlength=TB)
    CPB = int(math.ceil(counts.max() / P))
    ofs = np.zeros(TB + 1, np.int64)
    np.cumsum(counts, out=ofs[1:])
    pos_in_blk = np.arange(E) - ofs[eblk_s]
    chunk = pos_in_blk // P
    lane = pos_in_blk % P

    esrc_full = np.zeros((TB, CPB, P), np.int32)
    dstl_full = np.full((TB, CPB, P), PAD_LANE, np.float32)
    esrc_full[eblk_s, chunk, lane] = gid_of[src[eord]].astype(np.int32)
    dstl_full[eblk_s, chunk, lane] = (gdst[eord] & 127).astype(np.float32)

    return dict(
        NB=NB, CPB=CPB, NPC=NPC, NTOT=NTOT,
        dis=dis, gid_of=gid_of, node_of_gid=node_of_gid,
        esrc_full=esrc_full, dstl_full=dstl_full,
    )


def kernel(x, edge_index, W1, b1, W2, b2):
    x = np.ascontiguousarray(np.asarray(x, dtype=np.float32))
    W1 = np.ascontiguousarray(np.asarray(W1, dtype=np.float32))
    W2 = np.ascontiguousarray(np.asarray(W2, dtype=np.float32))
    b1 = np.asarray(b1, dtype=np.float32)
    b2 = np.asarray(b2, dtype=np.float32)
    src = np.asarray(edge_index[0]).astype(np.int64)
    dst = np.asarray(edge_index[1]).astype(np.int64)

    N, IN_CH = x.shape
    HID = W1.shape[1]
    OUT_CH = W2.shape[1]
    pr = _prepare(x, src, dst)
    NB, CPB, NPC = pr["NB"], pr["CPB"], pr["NPC"]
    node_of_gid = pr["node_of_gid"]
    dis = pr["dis"]

    has_b1 = bool(np.any(b1))
    has_b2 = bool(np.any(b2))
    nc = _build_program(NB, CPB, IN_CH, HID, OUT_CH, has_b1, has_b2)

    b1bc = np.ascontiguousarray(np.broadcast_to(b1, (P, HID)))
    b2bc = np.ascontiguousarray(np.broadcast_to(b2, (P, OUT_CH)))

    in_maps = []
    for c in range(NCORES):
        slots = node_of_gid[c * NPC : (c + 1) * NPC]  # [NPC] orig node or -1
        valid = slots >= 0
        xs = np.zeros((NPC, IN_CH), np.float32)
        xs[valid] = x[slots[valid]]
        xTc = np.ascontiguousarray(xs.T)
        disc = np.ones(NPC, np.float32)
        disc[valid] = dis[slots[valid]]
        disTc = np.ascontiguousarray(disc.reshape(NB, P).T)
        # [block, chunk, lane] -> [lane, block*CPB + chunk]
        esrcc = np.ascontiguousarray(
            pr["esrc_full"][c * NB : (c + 1) * NB].transpose(2, 0, 1).reshape(P, -1)
        )
        dstlc = np.ascontiguousarray(
            pr["dstl_full"][c * NB : (c + 1) * NB]
            .transpose(2, 0, 1)
            .reshape(P, -1)
        )
        in_maps.append(
            {
                "xT": xTc,
                "W1": W1,
                "W2": W2,
                "b1bc": b1bc,
                "b2bc": b2bc,
                "disT": disTc,
                "esrc": esrcc,
                "dstl": dstlc,
            }
        )

    res = run_bass_kernel_spmd(nc, in_maps, core_ids=list(range(NCORES)))
    global _last_results, _last_nc
    _last_results = res
    _last_nc = nc

    out = np.empty((N, OUT_CH), np.float32)
    for c in range(NCORES):
        oc = res.results[c]["outY"]
        slots = node_of_gid[c * NPC : (c + 1) * NPC]
        valid = slots >= 0
        out[slots[valid]] = oc[valid]
    return out
